# revision 20
# baseline (speedup 1.0000x reference)
r"""Circulant layer kernel for Trainium2 (8 NeuronCores).

Math: reference computes mv1 + mv2 where
  mv1 = batch_circulant(b) @ d,  mv2 = batch_circulant(d) @ b,
with d = des @ K, b = body @ K.  Both are the circular convolution of d and b
(circular convolution is commutative), so  out = 2 * circconv(d, b).

circconv via DFT:  out = 2 * Re(IDFT(DFT(d) * DFT(b))).
DFT/IDFT are realized as dense matmuls with host-generated constant
cos/sin matrices (input-independent constants).

Sharding: each of the 8 cores owns 128 of the 1024 DFT frequencies.
Per core c:
  KC_c   = K @ CC_c            (1024k x 256s)   fused projection+forward DFT
  DT_c   = KC_c^T @ des^T      (256s x 128b)    \  shares stationary weights
  BT_c   = KC_c^T @ body^T     (256s x 128b)    /
  PT_c   = complex-mult(DT_c, BT_c)             (256s x 128b)  on VectorE
  part_c = (PT_c^T @ G_c)                       (128b x 1024)  inverse DFT
Host sums the 8 partials (unshard).

Schedule: ktcc ships as 4 chunk DMAs (2 j-chunks each) on one serial SP
queue; stage-1 runs j-outer over the first 3 chunks so the PE chases the
DMA stream, then a kb-outer final phase (j=6,7 + stop) interleaves the
psum->bf16 casts and stage-2 accumulation per kb.  Pointwise reads the
stage-2 PSUM directly.  DMA order: ktcc0..2, dbt, ktcc3, g.
"""

import numpy as np

import concourse.bass as bass
import concourse.mybir as mybir
import concourse.tile as tile
from concourse.bass_utils import run_bass_kernel_spmd
from concourse.tile_rust import add_dep_helper

B = 128        # batch
D_IN = 1024    # input feature dim (contraction k)
N = 1024       # output feature dim (conv length j) == #frequencies
N_CORES = 8
FPC = N // N_CORES  # frequencies per core (complex)
S = 2 * FPC         # freq slots per core: [0:FPC]=real(cos), [FPC:2FPC]=imag(-sin)

F32 = mybir.dt.float32
F32R = mybir.dt.float32r
BF16 = mybir.dt.bfloat16

# Matmul operand precision: "bf16" (fastest; ~5e-3 rel err), "f32r"
# (single-pass TF32-like; ~3e-4), "f32" (two-pass full fp32; ~7e-7).
import os as _os
MM_PREC = _os.environ.get("CIRC_MM_PREC", "bf16")
MM_DT = {"bf16": BF16, "f32r": F32R, "f32": F32}[MM_PREC]


def _np_in(a):
    """Cast to the matmul precision; bf16 data is shipped packed in fp32
    words (DMA is element-rate-bound: 2-byte elements run at half rate)."""
    import ml_dtypes
    a = np.ascontiguousarray(np.asarray(a, dtype=np.float32))
    if MM_PREC != "bf16":
        return a
    bf = np.ascontiguousarray(a.astype(ml_dtypes.bfloat16))
    return bf.view(np.uint8).reshape(a.shape[0], -1).view(np.float32)

# Number of fp32 transport words per logical input element.
PACK = 2 if MM_PREC == "bf16" else 1
# Transport dtype: bf16 ships packed in fp32 words; f32/f32r ship natively
# (the fp32r verifier requires the producing DMA to be f32r-typed).
TR_DT = F32 if MM_PREC == "bf16" else MM_DT

# Stashed by kernel() for test harnesses that want profiling info.
LAST_RESULT = None

_nc_cache = {}

JC = N // 128      # 8 chunks over j (contraction of KC stage)
KB = D_IN // 128   # 8 blocks over k (output partitions of KC stage)
SB = S // 128      # 2 blocks over freq slots
NCHUNK = 4         # ktcc ships as 4 DMAs of 2 j-chunks each
WARM = 12          # PE warmup matmuls (HAM clock ramp)


def _build_nc():
    """Build the (single-program) Bass module run on all 8 cores."""
    nc = bass.Bass(target_bir_lowering=True)

    # Packed inputs, one DRAM tensor per serial-queue transfer:
    #   ktcc<c>[p] = rows p of j-chunks 2c,2c+1 of [K^T | CC]  (j indexes rows)
    #   dbt[p]  = rows p of the 8 k-chunks of [des^T | body^T]
    #   g[p]    = rows p of the SB s-chunks of G
    # All host-packed per SBUF partition: row p holds everything partition p
    # receives, contiguously, so each DMA moves 128 long contiguous rows.
    XW = (D_IN + S) // PACK
    DW = 2 * B // PACK
    GW = N // PACK
    ktcc_q = [nc.declare_dram_parameter(f"ktcc{i}", [128, 2 * XW], TR_DT, False)
              for i in range(NCHUNK)]
    dbt_q = nc.declare_dram_parameter("dbt", [128, KB * DW], TR_DT, False)
    g_q = nc.declare_dram_parameter("g", [128, SB * GW], TR_DT, False)
    out = nc.declare_dram_parameter("out", [B, N], F32, isOutput=True)

    with tile.TileContext(nc) as tc:
        with (
            tc.tile_pool(name="main", bufs=1) as pool,
            tc.tile_pool(name="psum", bufs=1, space="PSUM") as pp,
        ):
            # ---- inputs -> SBUF ----
            # All input transfers ride ONE serial SP chain: parallel channels
            # all pay the full proxy latency, while a serial chain pipelines.
            # Order = consumption order: ktcc0..2, dbt, ktcc3, g.
            in_dmas = []
            ktcc_sb = [pool.tile([128, 2, XW], TR_DT, tag=f"ktcc{q}", name=f"ktcc{q}")
                       for q in range(NCHUNK)]
            dbt_sb = pool.tile([128, KB, DW], TR_DT, tag="dbt", name="dbt")
            g_stage = pool.tile([128, SB, GW], TR_DT, tag="gst", name="gst")
            for q in range(3):
                in_dmas.append(nc.sync.dma_start(ktcc_sb[q][:], ktcc_q[q][:, :]))
            in_dmas.append(nc.sync.dma_start(dbt_sb[:], dbt_q[:, :]))
            in_dmas.append(nc.sync.dma_start(ktcc_sb[3][:], ktcc_q[3][:, :]))
            in_dmas.append(nc.sync.dma_start(g_stage[:], g_q[:, :]))
            ktcc_v = [t.bitcast(MM_DT) for t in ktcc_sb]
            kt_sb = [ktcc_v[j // 2][:, j % 2, :D_IN] for j in range(JC)]
            cc_sb = [ktcc_v[j // 2][:, j % 2, D_IN:] for j in range(JC)]
            dbt_v = dbt_sb.bitcast(MM_DT)
            g_sb = [g_stage.bitcast(MM_DT)[:, s, :] for s in range(SB)]

            # PSUM budget is 8 bank-tags, statically allocated per tag:
            #   kcp0..kcp4  (5) - stage-1 chase chains kb0..4, then kb5..7
            #   dbp0, dbp1  (2) - stage-2 accumulation chains
            #   op          (1) - stage-4 (h=0 fresh; h=1 reuses after copy)
            # PSUM accumulation chains are bank-granular (one live chain per
            # 2KB bank), so every concurrent chain needs its own tag.  A
            # start-matmul on a REUSED bank gets a walrus-injected drain
            # wait, so reused banks must need no other cross-engine wait
            # (no PE warmup, stage-4 h=0 on a fresh bank, late kb chains
            # gated only by their predecessor's cast).

            # ---- stage 1 (j-outer passes chase the DMA chunks) ----
            # KC[k, s] = sum_j KT[j, k] * CC[j, s].  kb0..4 accumulate in
            # parallel so each arriving ktcc chunk enables 10 matmuls
            # immediately; kb5..7 run whole chains in the final phase on
            # the banks kb0..2 free.
            NCH = 5
            ps = [pp.tile([128, S], F32, tag=f"kcp{kb}", name=f"kcp{kb}")
                  for kb in range(NCH)]
            kc_sb = [pool.tile([128, S], MM_DT, tag=f"kc{kb}", name=f"kc{kb}")
                     for kb in range(KB)]
            for p in range(3):
                for jj in range(2):
                    j = 2 * p + jj
                    for kb in range(NCH):
                        nc.tensor.matmul(
                            ps[kb][:],
                            kt_sb[j][:, kb * 128:(kb + 1) * 128],
                            cc_sb[j][:],
                            start=(j == 0),
                            stop=False,
                        )

            def finish_kb(pst, kb):
                nc.tensor.matmul(pst[:],
                                 kt_sb[6][:, kb * 128:(kb + 1) * 128],
                                 cc_sb[6][:], start=False, stop=False)
                nc.tensor.matmul(pst[:],
                                 kt_sb[7][:, kb * 128:(kb + 1) * 128],
                                 cc_sb[7][:], start=False, stop=True)
                nc.vector.tensor_copy(kc_sb[kb][:], pst[:])

            def stage2_kb(kb):
                for sb in range(SB):
                    nc.tensor.matmul(db_ps[sb][:, :2 * B],
                                     kc_sb[kb][:, sb * 128:(sb + 1) * 128],
                                     dbt_v[:, kb, :],
                                     start=(kb == 0), stop=(kb == KB - 1))

            # final phase: kb0..2 finish first (their casts free the banks
            # kb5..7 claim); the stage-2 chains fold each finished kb in
            # while later kbs still run.
            finish_kb(ps[0], 0)
            finish_kb(ps[1], 1)
            db_ps = [pp.tile([128, 512], F32, tag=f"dbp{sb}", name=f"dbp{sb}")
                     for sb in range(SB)]
            stage2_kb(0)
            stage2_kb(1)
            for kb in range(2, NCH):
                finish_kb(ps[kb], kb)
                stage2_kb(kb)
            for kb in range(NCH, KB):
                pst = pp.tile([128, S], F32, tag=f"kcp{kb - NCH}", name=f"kcp{kb}b")
                for j in range(6):
                    nc.tensor.matmul(pst[:],
                                     kt_sb[j][:, kb * 128:(kb + 1) * 128],
                                     cc_sb[j][:], start=(j == 0), stop=False)
                finish_kb(pst, kb)
                stage2_kb(kb)

            # ---- stage 3: complex pointwise multiply (on freq partitions) ----
            # Reads the stage-2 PSUM directly (no staging copy).
            # t01 = [Dr*Br, Dr*Bi], t23 = [Di*Bi, Di*Br]
            # Pr = t01[0] - t23[0],  Pi = t01[1] + t23[1]
            t01 = pool.tile([128, 2, B], F32, tag="t01", name="t01")
            t23 = pool.tile([128, 2, B], F32, tag="t23", name="t23")
            pt = pool.tile([128, 2, B], MM_DT, tag="pt", name="pt")
            # Vector ops may read only ONE operand from PSUM: stage the
            # B-halves in SBUF, read the D-halves straight from PSUM.
            bb = pool.tile([128, 2, B], F32, tag="bb", name="bb")
            nc.vector.tensor_copy(bb[:, 0, :], db_ps[0][:, B:2 * B])
            nc.vector.tensor_copy(bb[:, 1, :], db_ps[1][:, B:2 * B])
            dr_b = db_ps[0][:, :B][:, None, :].to_broadcast((128, 2, B))
            di_b = db_ps[1][:, :B][:, None, :].to_broadcast((128, 2, B))
            nc.vector.tensor_mul(t01[:], dr_b, bb[:])
            nc.vector.tensor_mul(t23[:], di_b, bb[:, ::-1, :])
            nc.vector.tensor_sub(pt[:, 0, :], t01[:, 0, :], t23[:, 0, :])
            nc.vector.tensor_add(pt[:, 1, :], t01[:, 1, :], t23[:, 1, :])
            pt_sb = [pt[:, sb, :] for sb in range(SB)]

            # ---- stage 4: part = PT^T @ G ----
            out_sb = pool.tile([128, N], F32, tag="outsb", name="outsb")
            last_mm = last_cp = None
            for h in range(2):
                o_ps = pp.tile([128, 512], F32, tag="op", name=f"op{h}")
                for sb in range(SB):
                    last_mm = nc.tensor.matmul(
                        o_ps[:],
                        pt_sb[sb],
                        g_sb[sb][:, h * 512:(h + 1) * 512],
                        start=(sb == 0),
                        stop=(sb == SB - 1),
                    )
                last_cp = nc.vector.tensor_copy(out_sb[:, h * 512:(h + 1) * 512], o_ps[:])
            store_a = nc.sync.dma_start(out[:, :512], out_sb[:, :512])
            store_b = nc.scalar.dma_start(out[:, 512:], out_sb[:, 512:])

            # TileContext's exit emits one tail Drain waiting on every
            # outstanding semaphore; walrus caps instructions at ONE sync
            # wait.  Pre-absorb every tick into SP's clock with a chain of
            # single-wait drains so the tail drain needs none.
            prev = None
            for dep in [*in_dmas, store_a, store_b, last_mm, last_cp]:
                dr = nc.sync.drain(fusable=False)
                add_dep_helper(dr.ins, dep.ins, sync=True,
                               reason="tail: absorb tick into SP clock")
                if prev is not None:
                    add_dep_helper(dr.ins, prev.ins, sync=False,
                                   reason="tail: keep drain chain ordered")
                prev = dr

    return nc


def _dft_constants():
    """Per-core forward (CC) and inverse (G) DFT matrices, float32."""
    j = np.arange(N, dtype=np.float64)
    ccs, gs = [], []
    for c in range(N_CORES):
        f = np.arange(c * FPC, (c + 1) * FPC, dtype=np.float64)
        ang = 2.0 * np.pi * np.outer(j, f) / N          # (j, f)
        cc = np.concatenate([np.cos(ang), -np.sin(ang)], axis=1)   # (N, S)
        # inverse: out[k] = (2/N) * sum_f [Pr cos(2pi f k/N) - Pi sin(2pi f k/N)]
        angT = ang.T                                     # (f, k)
        gr = (2.0 / N) * np.cos(angT)
        gi = -(2.0 / N) * np.sin(angT)
        gmat = np.concatenate([gr, gi], axis=0)          # (S, N)
        ccs.append(np.ascontiguousarray(cc, dtype=np.float32))
        gs.append(np.ascontiguousarray(gmat, dtype=np.float32))
    return ccs, gs


def _partition_pack(a):
    """(R, W) with R = n*128 -> (128, n*W): row p = concat of chunk rows p."""
    r, w = a.shape
    n = r // 128
    return np.ascontiguousarray(
        a.reshape(n, 128, w).transpose(1, 0, 2).reshape(128, n * w))


def kernel(des, body, kernel):
    global LAST_RESULT
    K = np.asarray(kernel, dtype=np.float32)
    kt_np = K.T  # (j, k)
    dbt_np = _partition_pack(_np_in(np.concatenate(
        [np.asarray(des, dtype=np.float32).T, np.asarray(body, dtype=np.float32).T],
        axis=1,
    )))  # (k, 2B) packed
    ccs, gs = _dft_constants()
    ktccs = [
        _partition_pack(_np_in(np.concatenate([kt_np, ccs[c]], axis=1)))
        for c in range(N_CORES)
    ]
    quarter = ktccs[0].shape[1] // NCHUNK
    gpacked = [_partition_pack(_np_in(gs[c])) for c in range(N_CORES)]

    if "nc" not in _nc_cache:
        _nc_cache["nc"] = _build_nc()
    nc = _nc_cache["nc"]

    in_maps = [
        {**{f"ktcc{i}": np.ascontiguousarray(
                ktccs[c][:, i * quarter:(i + 1) * quarter])
            for i in range(NCHUNK)},
         "dbt": dbt_np, "g": gpacked[c]}
        for c in range(N_CORES)
    ]
    res = run_bass_kernel_spmd(nc, in_maps, list(range(N_CORES)))
    LAST_RESULT = res
    out = np.zeros((B, N), dtype=np.float32)
    for r in res.results:
        out += r["out"]
    return out
